# revision 1
# baseline (speedup 1.0000x reference)
import numpy as np
import ml_dtypes

import concourse.bass as bass
import concourse.bacc as bacc
import concourse.mybir as mybir
from concourse.tile import TileContext
from concourse.bass_utils import run_bass_kernel_spmd

B, DIM, L = 16, 1024, 4096
HEADS, DH = 16, 64
INNER = HEADS * DH
TOPK = 64
NCORES = 8
BPC = B // NCORES  # batches per core = 2
KC = DIM // 128    # 8 contraction chunks
NT = L // 512      # 8 N tiles
BF16 = mybir.dt.bfloat16
F32 = mybir.dt.float32

_CACHE = {}


def _build_mm_kernel():
    """Kernel A: per core, for 2 batches: kv = Wkv @ xc, q = Wq @ xq.

    Inputs (bf16): xc (2,1024,4096), xq (2,1024,4096), wkvt (1024,2048),
    wqt (1024,1024). Outputs fp32: kv (2,2048,4096), q (2,1024,4096).
    """
    nc = bacc.Bacc(None, target_bir_lowering=False)
    xc = nc.dram_tensor("xc", [BPC, DIM, L], BF16, kind="ExternalInput")
    xq = nc.dram_tensor("xq", [BPC, DIM, L], BF16, kind="ExternalInput")
    wkvt = nc.dram_tensor("wkvt", [DIM, 2 * INNER], BF16, kind="ExternalInput")
    wqt = nc.dram_tensor("wqt", [DIM, INNER], BF16, kind="ExternalInput")
    kv = nc.dram_tensor("kv", [BPC, 2 * INNER, L], F32, kind="ExternalOutput")
    q = nc.dram_tensor("q", [BPC, INNER, L], F32, kind="ExternalOutput")

    with TileContext(nc) as tc:
        with (
            tc.tile_pool(name="w", bufs=8) as wp,
            tc.tile_pool(name="x", bufs=8) as xp,
            tc.tile_pool(name="ps", bufs=8, space="PSUM") as pp,
            tc.tile_pool(name="st", bufs=4) as sp,
        ):
            # weights resident
            wkvt_sb = [wp.tile([128, 2 * INNER], BF16, tag="wkv", name=f"wkv{_}") for _ in range(KC)]
            wqt_sb = [wp.tile([128, INNER], BF16, tag="wq", name=f"wq{_}") for _ in range(KC)]
            for kc in range(KC):
                nc.sync.dma_start(out=wkvt_sb[kc], in_=wkvt[kc * 128:(kc + 1) * 128, :])
                nc.sync.dma_start(out=wqt_sb[kc], in_=wqt[kc * 128:(kc + 1) * 128, :])

            for b in range(BPC):
                for (x_dram, w_sb, out_dram, mtiles) in (
                    (xc, wkvt_sb, kv, 2 * INNER // 128),
                    (xq, wqt_sb, q, INNER // 128),
                ):
                    x_sb = [xp.tile([128, L], BF16, tag="xin", name=f"xin{_}") for _ in range(KC)]
                    for kc in range(KC):
                        nc.sync.dma_start(
                            out=x_sb[kc], in_=x_dram[b, kc * 128:(kc + 1) * 128, :])
                    for mt in range(mtiles):
                        for nt in range(NT):
                            ps = pp.tile([128, 512], F32, name="ps")
                            for kc in range(KC):
                                nc.tensor.matmul(
                                    out=ps,
                                    lhsT=w_sb[kc][:, mt * 128:(mt + 1) * 128],
                                    rhs=x_sb[kc][:, nt * 512:(nt + 1) * 512],
                                    start=(kc == 0), stop=(kc == KC - 1))
                            st = sp.tile([128, 512], F32, tag="stage", name="stage")
                            nc.scalar.copy(out=st, in_=ps)
                            nc.gpsimd.dma_start(
                                out=out_dram[b, mt * 128:(mt + 1) * 128,
                                             nt * 512:(nt + 1) * 512],
                                in_=st)
    nc.finalize()
    return nc


def _build_out_kernel(gamma: float):
    """Kernel B: final = gamma * (W_out @ ao) + qs_raw."""
    nc = bacc.Bacc(None, target_bir_lowering=False)
    ao = nc.dram_tensor("ao", [BPC, INNER, L], BF16, kind="ExternalInput")
    qs = nc.dram_tensor("qs", [BPC, DIM, L], F32, kind="ExternalInput")
    woutt = nc.dram_tensor("woutt", [INNER, DIM], BF16, kind="ExternalInput")
    fin = nc.dram_tensor("fin", [BPC, DIM, L], F32, kind="ExternalOutput")

    with TileContext(nc) as tc:
        with (
            tc.tile_pool(name="w", bufs=8) as wp,
            tc.tile_pool(name="x", bufs=8) as xp,
            tc.tile_pool(name="r", bufs=3) as rp,
            tc.tile_pool(name="ps", bufs=8, space="PSUM") as pp,
            tc.tile_pool(name="st", bufs=4) as sp,
        ):
            w_sb = [wp.tile([128, DIM], BF16, tag="w", name=f"w{_}") for _ in range(KC)]
            for kc in range(KC):
                nc.sync.dma_start(out=w_sb[kc], in_=woutt[kc * 128:(kc + 1) * 128, :])
            for b in range(BPC):
                x_sb = [xp.tile([128, L], BF16, tag="xin", name=f"xin{_}") for _ in range(KC)]
                for kc in range(KC):
                    nc.sync.dma_start(
                        out=x_sb[kc], in_=ao[b, kc * 128:(kc + 1) * 128, :])
                for mt in range(DIM // 128):
                    for nt in range(NT):
                        ps = pp.tile([128, 512], F32, name="ps")
                        for kc in range(KC):
                            nc.tensor.matmul(
                                out=ps,
                                lhsT=w_sb[kc][:, mt * 128:(mt + 1) * 128],
                                rhs=x_sb[kc][:, nt * 512:(nt + 1) * 512],
                                start=(kc == 0), stop=(kc == KC - 1))
                        res = rp.tile([128, 512], F32, tag="res", name="res")
                        nc.sync.dma_start(
                            out=res,
                            in_=qs[b, mt * 128:(mt + 1) * 128,
                                   nt * 512:(nt + 1) * 512])
                        st = sp.tile([128, 512], F32, tag="stage", name="stage")
                        nc.vector.scalar_tensor_tensor(
                            out=st, in0=ps, scalar=float(gamma),
                            op0=mybir.AluOpType.mult, in1=res,
                            op1=mybir.AluOpType.add)
                        nc.gpsimd.dma_start(
                            out=fin[b, mt * 128:(mt + 1) * 128,
                                    nt * 512:(nt + 1) * 512],
                            in_=st)
    nc.finalize()
    return nc


def _bf16(x):
    return np.asarray(x, np.float32).astype(ml_dtypes.bfloat16)


def _run(nc, in_maps):
    res = run_bass_kernel_spmd(nc, in_maps, list(range(NCORES)))
    return res.results


def kernel(context, query_source, gamma_c, beta_c, gamma_q, beta_q,
           W_kv, W_q, W_out, gamma):
    context = np.asarray(context, np.float32)
    query_source = np.asarray(query_source, np.float32)
    W_kv = np.asarray(W_kv, np.float32)
    W_q = np.asarray(W_q, np.float32)
    W_out = np.asarray(W_out, np.float32)
    g = float(np.asarray(gamma).reshape(-1)[0])

    def chan_norm(x, gam, bet):
        mean = x.mean(axis=1, keepdims=True, dtype=np.float32)
        var = x.var(axis=1, keepdims=True, dtype=np.float32)
        return (np.asarray(gam, np.float32) * (x - mean) /
                (np.sqrt(var) + 1e-6) + np.asarray(bet, np.float32))

    ctx_n = chan_norm(context, gamma_c, beta_c)
    qs_n = chan_norm(query_source, gamma_q, beta_q)

    if "mm" not in _CACHE:
        _CACHE["mm"] = _build_mm_kernel()
    nc_a = _CACHE["mm"]
    wkvt = _bf16(W_kv.T)
    wqt = _bf16(W_q.T)
    in_maps = []
    for c in range(NCORES):
        sl = slice(c * BPC, (c + 1) * BPC)
        in_maps.append({
            "xc": _bf16(ctx_n[sl]), "xq": _bf16(qs_n[sl]),
            "wkvt": wkvt, "wqt": wqt,
        })
    res_a = _run(nc_a, in_maps)
    kv = np.concatenate([r["kv"] for r in res_a], axis=0)  # (B, 2048, L)
    q = np.concatenate([r["q"] for r in res_a], axis=0)    # (B, 1024, L)

    # host: fold heads, l2norm, probe topk, gather, attention
    def fold(t):
        return t.reshape(B, HEADS, -1, L).reshape(B * HEADS, -1, L)

    k, v = np.split(kv, 2, axis=1)
    q = fold(q)
    k = fold(k)
    v = fold(v)

    def l2n(x):
        n = np.sqrt(np.sum(x * x, axis=1, keepdims=True))
        return x / np.maximum(n, 1e-12)

    q = l2n(q)
    k = l2n(k)
    qp = np.abs(q).sum(axis=2)                       # (BH, DH)
    score = np.einsum("bc,bcl->bl", qp, np.abs(k))   # (BH, L)
    top_idx = np.argpartition(score, L - TOPK, axis=1)[:, L - TOPK:]
    idx3 = np.broadcast_to(top_idx[:, None, :], (B * HEADS, DH, TOPK))
    k_sel = np.take_along_axis(k, idx3, axis=2)      # (BH, DH, TOPK)
    v_sel = np.take_along_axis(v, idx3, axis=2)

    qt = np.ascontiguousarray(q.transpose(0, 2, 1))  # (BH, L, DH)
    sim = np.matmul(qt, k_sel)                       # (BH, L, TOPK)
    sim -= sim.max(axis=-1, keepdims=True)
    e = np.exp(sim, dtype=np.float32)
    attn = e / e.sum(axis=-1, keepdims=True)
    out = np.matmul(attn, v_sel.transpose(0, 2, 1))  # (BH, L, DH)
    ao = out.reshape(B, HEADS, L, DH).transpose(0, 1, 3, 2).reshape(B, INNER, L)

    key_b = ("out", g)
    if key_b not in _CACHE:
        _CACHE[key_b] = _build_out_kernel(g)
    nc_b = _CACHE[key_b]
    woutt = _bf16(W_out.T)
    in_maps_b = []
    for c in range(NCORES):
        sl = slice(c * BPC, (c + 1) * BPC)
        in_maps_b.append({
            "ao": _bf16(ao[sl]),
            "qs": np.ascontiguousarray(query_source[sl]),
            "woutt": woutt,
        })
    res_b = _run(nc_b, in_maps_b)
    fin = np.concatenate([r["fin"] for r in res_b], axis=0)
    return fin.astype(np.float32)

